# revision 25
# baseline (speedup 1.0000x reference)
"""MelBandSplit Trainium2 kernel.

Problem (hardcoded): x [B=4, C=4, T=512, F=1025] fp32.
  xr = x.transpose(0,2,3,1).reshape(B, T, F*C)   # [B, T, 4100]
  For each of 61 bands (feature ranges of width d in {32,64,128,256,4}):
    seg = xr[..., f0:f0+d]
    seg = seg * (sqrt(d) / max(||seg||, 1e-12)) * gamma
    out_b = seg @ W_b.T + bias_b                  # FEAT=512
  out = stack over bands -> [B, T, 61, 512]

Sharding: data-parallel over B*T rows; 2048 rows -> 256 rows/core on 8 cores.
gamma and sqrt(d) are folded into the weights on the host; the per-(row,band)
1/max(norm,eps) factor is applied on-chip when draining PSUM.

Device layout per core:
  xt_sb  [128, 32, 256]  : x transposed (feature on partitions) + [4,256] tail
  wt_sb  [128, 32, 512]  : W^T (feature on partitions) + [4,512] tail
  xrp    [128, <=2048]   : rotating row-major pieces (for sum-of-squares)
  matmul: psum[rows=128, 512] += xt_chunk[K, 128 rows].T @ wt_chunk[K, 512]
  drain : stage = psum * (1/max(nrm, 1e-12)) on DVE/ACT, then DMA to DRAM

Timing on this box: ~135us/core (range 133-153 run-to-run); DMA-bound —
48.7MB/core moved serially through the strictly-arbitrated HWDGE queue at
~360GB/s. fp32 matmuls (2 half-rate passes) keep PE busy ~112us, just
under the DMA span.
"""

import math
from contextlib import ExitStack

import numpy as np

CH = 4
FEAT = 512
B, T, F = 4, 512, 1025
NCORES = 8
ROWS = B * T          # 2048
RPC = ROWS // NCORES  # 256 rows per core
DTOT = F * CH         # 4100
NT = DTOT // 128      # 32 full 128-feature tiles (+ tail of 4)
TAIL = DTOT - NT * 128  # 4


def _band_ranges():
    br, lo = [], 0
    for w, n in [(8, 32), (16, 16), (32, 8), (64, 4), (1, 1)]:
        for _ in range(n):
            br.append((lo, lo + w - 1))
            lo += w
    return br


BAND_RANGES = _band_ranges()
NBANDS = len(BAND_RANGES)  # 61
# (feature offset, feature count) per band in the flattened F*C dim
BANDS = [(lo * CH, (hi - lo + 1) * CH) for lo, hi in BAND_RANGES]

GROUP = 8  # bands per output staging buffer / store DMA

_CACHE = {}


def _build_bass():
    import concourse.tile as tile
    from concourse import bacc, mybir

    f32 = mybir.dt.float32
    nc = bacc.Bacc()
    xr_d = nc.declare_dram_parameter("xr", [RPC, DTOT], f32, isOutput=False)
    xt_d = nc.declare_dram_parameter("xt", [DTOT, RPC], f32, isOutput=False)
    wt_d = nc.declare_dram_parameter("wt", [DTOT, FEAT], f32, isOutput=False)
    out_d = nc.declare_dram_parameter("out", [RPC, NBANDS, FEAT], f32, isOutput=True)

    # Load chunks of 128-feature tiles, small first so the first matmuls
    # start after ~1.5MB of loads. All loads are traced up front: on the
    # single SP HWDGE FIFO they then stream at full rate ahead of stores,
    # and the deep stage pool absorbs the store latency.
    CHUNKS = [(0, 2), (2, 2), (4, 4), (8, 4), (12, 4), (16, 4), (20, 4),
              (24, 4), (28, 4)]
    PIECES = [
        (0, 32, [(0, 32, 32)]),
        (32, 24, [(32, 16, 64), (48, 8, 128)]),
        (56, 5, [(56, 4, 256), (60, 1, 4)]),
    ]

    with ExitStack() as ctx:
        tc = ctx.enter_context(tile.TileContext(nc))
        persist = ctx.enter_context(tc.tile_pool(name="persist", bufs=1))
        xrp_pool = ctx.enter_context(tc.tile_pool(name="xrp", bufs=2))
        stage_pool = ctx.enter_context(tc.tile_pool(name="stage", bufs=5))
        psum_pool = ctx.enter_context(tc.tile_pool(name="psum", bufs=8, space="PSUM"))

        xt_sb = persist.tile([128, NT, RPC], f32)
        wt_sb = persist.tile([128, NT, FEAT], f32)
        xt_tail = persist.tile([TAIL, RPC], f32)
        wt_tail = persist.tile([TAIL, FEAT], f32)
        ss_t = [persist.tile([128, NBANDS], f32, name=f"ss{r}") for r in range(2)]
        scales = [persist.tile([128, NBANDS], f32, name=f"inv{r}") for r in range(2)]

        def load_piece(pi):
            # Load an xr piece into a rotating buffer, square it, segmented
            # reduce, then sqrt/clip/recip into scales.
            b0, nbp, runs = PIECES[pi]
            f0 = BANDS[b0][0]
            fend = BANDS[b0 + nbp - 1][0] + BANDS[b0 + nbp - 1][1]
            for r in range(2):
                xrp = xrp_pool.tile([128, 2048], f32, name="xrp", tag="xrp")
                nc.sync.dma_start(
                    out=xrp[:, : fend - f0],
                    in_=xr_d[r * 128 : (r + 1) * 128, f0:fend],
                )
                nc.scalar.activation(
                    out=xrp[:, : fend - f0],
                    in_=xrp[:, : fend - f0],
                    func=mybir.ActivationFunctionType.Square,
                )
                ss = ss_t[r]
                for rb0, rnb, w in runs:
                    rf0 = BANDS[rb0][0] - f0
                    nc.vector.reduce_sum(
                        out=ss[:, rb0 : rb0 + rnb],
                        in_=xrp[:, rf0 : rf0 + rnb * w].rearrange(
                            "p (b w) -> p b w", w=w
                        ),
                        axis=mybir.AxisListType.X,
                    )
                sl = slice(b0, b0 + nbp)
                nc.scalar.sqrt(out=ss[:, sl], in_=ss[:, sl])
                nc.vector.tensor_scalar_max(
                    out=ss[:, sl], in0=ss[:, sl], scalar1=1e-12
                )
                nc.vector.reciprocal(out=scales[r][:, sl], in_=ss[:, sl])

        def load_chunk(ci):
            t0, tpc = CHUNKS[ci]
            nc.sync.dma_start(
                out=xt_sb[:, t0 : t0 + tpc, :],
                in_=xt_d[t0 * 128 : (t0 + tpc) * 128, :].rearrange(
                    "(t p) m -> p t m", p=128
                ),
            )
            nc.sync.dma_start(
                out=wt_sb[:, t0 : t0 + tpc, :],
                in_=wt_d[t0 * 128 : (t0 + tpc) * 128, :].rearrange(
                    "(t p) m -> p t m", p=128
                ),
            )

        load_chunk(0)
        load_chunk(1)
        load_piece(0)
        load_chunk(2)
        load_chunk(3)
        load_piece(1)
        load_chunk(4)
        load_chunk(5)
        load_piece(2)
        load_chunk(6)
        load_chunk(7)
        load_chunk(8)
        nc.sync.dma_start(out=xt_tail, in_=xt_d[NT * 128 : DTOT, :])
        nc.sync.dma_start(out=wt_tail, in_=wt_d[NT * 128 : DTOT, :])

        # ---- per-band matmul + scaled drain + store ----
        drain_idx = 0
        for g0 in range(0, NBANDS, GROUP):
            nb = min(GROUP, NBANDS - g0)
            for r in range(2):
                stage = stage_pool.tile([128, GROUP, FEAT], f32, name="stage", tag="st")
                for j in range(nb):
                    b = g0 + j
                    f0, d = BANDS[b]
                    psum = psum_pool.tile([128, FEAT], f32, name="ps", tag="ps")
                    nchunks = (d + 127) // 128
                    for ci in range(nchunks):
                        off = f0 + ci * 128
                        sz = min(128, d - ci * 128)
                        t, p0 = off // 128, off % 128
                        if t < NT:
                            lhsT = xt_sb[p0 : p0 + sz, t, r * 128 : (r + 1) * 128]
                            rhs = wt_sb[p0 : p0 + sz, t, :]
                        else:
                            lhsT = xt_tail[:, r * 128 : (r + 1) * 128]
                            rhs = wt_tail[:, :]
                        nc.tensor.matmul(
                            psum,
                            lhsT,
                            rhs,
                            start=(ci == 0),
                            stop=(ci == nchunks - 1),
                            tile_position=(p0, 0) if p0 else None,
                        )
                    if drain_idx % 2 == 1:
                        nc.scalar.mul(
                            out=stage[:, j, :], in_=psum, mul=scales[r][:, b : b + 1]
                        )
                    else:
                        nc.vector.tensor_scalar_mul(
                            out=stage[:, j, :], in0=psum,
                            scalar1=scales[r][:, b : b + 1],
                        )
                    drain_idx += 1
                nc.sync.dma_start(
                    out=out_d[r * 128 : (r + 1) * 128, g0 : g0 + nb, :],
                    in_=stage[:, :nb, :],
                )
    nc.compile()
    return nc


def _get_nc():
    if "nc" not in _CACHE:
        _CACHE["nc"] = _build_bass()
    return _CACHE["nc"]


def _prepare(inputs):
    x = np.ascontiguousarray(np.asarray(inputs["x"], dtype=np.float32))
    assert x.shape == (B, CH, T, F), x.shape
    gammas = [np.asarray(g, dtype=np.float32) for g in inputs["gammas"]]
    Ws = [np.asarray(w, dtype=np.float32) for w in inputs["Ws"]]
    bs = [np.asarray(bb, dtype=np.float32) for bb in inputs["bs"]]

    xr = np.ascontiguousarray(
        x.transpose(0, 2, 3, 1).reshape(ROWS, DTOT)
    )  # [2048, 4100]

    # Fold gamma and sqrt(d) into the weights: W' = W * (gamma * sqrt(d))
    wt = np.empty((DTOT, FEAT), dtype=np.float32)
    for (f0, d), g, W in zip(BANDS, gammas, Ws):
        wt[f0 : f0 + d, :] = (W * (g * math.sqrt(d))[None, :]).T
    wt = np.ascontiguousarray(wt)

    in_maps = []
    for c in range(NCORES):
        shard = xr[c * RPC : (c + 1) * RPC]
        in_maps.append(
            {
                "xr": np.ascontiguousarray(shard),
                "xt": np.ascontiguousarray(shard.T),
                "wt": wt,
            }
        )
    return in_maps, bs


def _run(inputs, trace=False):
    from concourse.bass_utils import run_bass_kernel_spmd

    in_maps, bs = _prepare(inputs)
    nc = _get_nc()
    res = run_bass_kernel_spmd(
        nc, in_maps, core_ids=list(range(NCORES)), trace=trace
    )
    out = np.concatenate([r["out"] for r in res.results], axis=0)
    out = out.reshape(B, T, NBANDS, FEAT)
    bias = np.stack(bs, axis=0)  # [61, 512]
    if np.any(bias):
        out = out + bias[None, None, :, :]
    return out, res


def kernel(**inputs) -> np.ndarray:
    out, _ = _run(inputs, trace=False)
    return out
